# revision 1
# baseline (speedup 1.0000x reference)
"""Chamfer distance kernel for Trainium2 (8 NeuronCores, data-parallel over batch).

Full inputs x, y: [8, 4096, 3] fp32. Output: [8] fp32.

Strategy (per core = one batch):
  dist(i,j) = ||x_i||^2 + ||y_j||^2 - 2 x_i.y_j  computed on the PE as a
  K=24 bf16 matmul using 3-way bf16 splits of coordinates and norms
  (fp32-equivalent: ~1e-5 abs error on distances).  Both min directions
  run the same pipeline with the two matmul layouts (x-on-partitions and
  y-on-partitions, swapping lhsT/rhs roles of the same two K-matrices).
  Each [128, 4096] row-block lands in PSUM as four 2-bank groups and is
  consumed by:
    - ScalarE copies of the odd groups PSUM->SBUF (parallel bank access)
    - VectorE tensor_tensor_scan(min, min): one op consumes a PSUM group
      plus a copied SBUF group; the scan's last column (chained via
      `initial`) is the per-point running min -> per-chunk min column.
  GpSimd extracts the scan tails; VectorE reduces columns to per-partition
  row-sums; host sums 128 values per direction and divides by n.
"""

import os
import sys

import numpy as np

for _p in ("/opt/trn_rl_repo", "/root/.axon_site/_ro/trn_rl_repo"):
    if os.path.isdir(_p) and _p not in sys.path:
        sys.path.insert(0, _p)

B = 8
N = 4096
D = 3
P = 128
IPP = N // P  # 32 points per partition
K = 24        # contraction rows
NCH = N // P  # 32 lhsT chunks of 128 points
HBANK = 1024  # free elems per 2-bank psum group
BIG = 3.0e38
MODE = os.environ.get("CHAMFER_MODE", "tts")  # "tts" | "tts2k" | "reduce" | "ttr"

_CACHE = {}


def _build_nc():
    from contextlib import ExitStack

    from concourse import bacc, mybir
    from concourse.tile import TileContext

    f32 = mybir.dt.float32
    bf16 = mybir.dt.bfloat16
    MIN = mybir.AluOpType.min
    ADD = mybir.AluOpType.add
    AX = mybir.AxisListType.X

    nc = bacc.Bacc()
    x_d = nc.declare_dram_parameter("x", [N, D], f32, isOutput=False)
    y_d = nc.declare_dram_parameter("y", [N, D], f32, isOutput=False)
    res_d = nc.declare_dram_parameter("res", [P, 2], f32, isOutput=True)

    with ExitStack() as ctx:
        tc = ctx.enter_context(TileContext(nc))
        singles = ctx.enter_context(tc.tile_pool(name="singles", bufs=1))
        copies = ctx.enter_context(tc.tile_pool(name="copies", bufs=3))
        scratch = ctx.enter_context(tc.tile_pool(name="scratch", bufs=2))

        # ---------- operand prep: K-matrices KM[side] of shape [24, 4096] bf16
        # Row pairing (lhs row r multiplies rhs row r, summed over r):
        #   r0-2 : 2*xh_c * -yh_c     r3-5 : 2*xh_c * -ym_c
        #   r6-8 : 2*xm_c * -yh_c    r9-11: 2*xm_c * -ym_c
        #   r12-14: 2*xh_c * -yl_c    r15-17: 2*xl_c * -yh_c
        #   r18-20: nx{h,m,l} * 1     r21-23: 1 * ny{h,m,l}
        # Built by assembling W[side] = [128 pts, 32 blocks, 32 rows] bf16 and
        # PE-transposing each [128, 32] block into KM[24, block*128:...].
        from concourse import masks

        # Warmup: dependency-free first ops per engine so one-time costs
        # (ACT table load ~2.7us, GpSimd library load) overlap the input DMAs
        # instead of sitting in the splits dependency chain.
        warm = singles.tile([P, 8], f32, tag="warm")
        nc.vector.memset(warm[:, 0:4], 0.0)
        nc.scalar.copy(warm[:, 4:6], warm[:, 0:2])
        nc.gpsimd.tensor_copy(warm[:, 6:8], warm[:, 2:4])

        ident = singles.tile([P, P], bf16, tag="ident")
        masks.make_identity(nc, ident[:])

        KM = {}
        WS = {}
        for side, dram in (("y", y_d), ("x", x_d)):
            sc = 2.0 if side == "x" else -1.0
            raw = singles.tile([P, IPP, D], f32, tag=f"raw_{side}")
            nc.sync.dma_start(
                out=raw[:],
                in_=dram[:, :].rearrange("(p i) c -> p i c", p=P),
            )
            # 3-way bf16 split of coordinates (natural layout).
            # Casts/subs go to ScalarE/GpSimd to keep VectorE (the main-loop
            # bottleneck engine) free.
            h = singles.tile([P, IPP, D], bf16, tag=f"h_{side}")
            nc.scalar.copy(h[:], raw[:])
            e1 = singles.tile([P, IPP, D], f32, tag=f"e1_{side}")
            sub_eng = nc.vector if side == "y" else nc.gpsimd
            sub_eng.tensor_sub(e1[:], raw[:], h[:])
            m = singles.tile([P, IPP, D], bf16, tag=f"m_{side}")
            nc.scalar.copy(m[:], e1[:])
            e2 = singles.tile([P, IPP, D], f32, tag=f"e2_{side}")
            sub_eng.tensor_sub(e2[:], e1[:], m[:])
            low = singles.tile([P, IPP, D], bf16, tag=f"l_{side}")
            nc.scalar.copy(low[:], e2[:])
            # norms + 3-way split
            sq = singles.tile([P, IPP, D], f32, tag=f"sq_{side}")
            nc.gpsimd.tensor_mul(sq[:], raw[:], raw[:])
            nrm = singles.tile([P, IPP], f32, tag=f"nrm_{side}")
            nc.vector.tensor_reduce(nrm[:], sq[:], axis=AX, op=ADD)
            nh = singles.tile([P, IPP], bf16, tag=f"nh_{side}")
            nc.scalar.copy(nh[:], nrm[:])
            ne1 = singles.tile([P, IPP], f32, tag=f"ne1_{side}")
            nc.gpsimd.tensor_sub(ne1[:], nrm[:], nh[:])
            nm = singles.tile([P, IPP], bf16, tag=f"nm_{side}")
            nc.scalar.copy(nm[:], ne1[:])
            ne2 = singles.tile([P, IPP], f32, tag=f"ne2_{side}")
            nc.gpsimd.tensor_sub(ne2[:], ne1[:], nm[:])
            nl = singles.tile([P, IPP], bf16, tag=f"nl_{side}")
            nc.scalar.copy(nl[:], ne2[:])

            # staging tile W: [128 pts-in-block, 32 blocks, 32 rows] bf16
            w = singles.tile([P, IPP, 32], bf16, tag=f"w_{side}")
            nc.gpsimd.memset(w[:], 1.0)  # rows 21-23 / 18-20 stay ones; 24-31 pad
            if side == "x":
                rowsrc = [(h, sc), (h, sc), (m, sc), (m, sc), (h, sc), (low, sc)]
                norm0 = 18
            else:
                rowsrc = [(h, sc), (m, sc), (h, sc), (m, sc), (low, sc), (h, sc)]
                norm0 = 21
            # y-side assembly on VectorE (idle during startup, gates main
            # start); x-side on GpSimd (VectorE is busy once main runs).
            asm = nc.vector if side == "y" else nc.gpsimd
            for g, (arr, s) in enumerate(rowsrc):
                for c in range(D):
                    r = 3 * g + c
                    asm.tensor_scalar_mul(w[:, :, r], arr[:, :, c], s)
            for c, arr in enumerate((nh, nm, nl)):
                asm.tensor_copy(w[:, :, norm0 + c], arr[:])

            km = singles.tile([32, N], bf16, tag=f"km_{side}")
            KM[side] = km
            WS[side] = w

        # ---------- main: two layouts, each 32 chunks x 2 half-rows
        psum_bufs = 1 if MODE == "tts2k" else 2
        psum = ctx.enter_context(
            tc.tile_pool(name="psum", bufs=psum_bufs, space="PSUM")
        )

        def emit_km_block(side, t4):
            # transpose 4 blocks into one PSUM bank, then copy out [32, 512]
            w = WS[side]
            km = KM[side]
            borrow = "pa" if MODE == "tts2k" else "p1"
            pt = psum.tile([32, 512], bf16, tag=borrow)  # borrow main psum slots
            for u in range(4):
                t = t4 * 4 + u
                nc.tensor.transpose(
                    pt[:, u * P : (u + 1) * P], w[:, t, :], ident[:]
                )
            if side == "y":
                nc.vector.tensor_copy(km[:, t4 * 512 : (t4 + 1) * 512], pt[:])
            else:
                nc.scalar.copy(km[:, t4 * 512 : (t4 + 1) * 512], pt[:])

        # y-side KM needed in full before the first matmul; x-side KM blocks
        # are emitted just-in-time inside the layout-A chunk loop.
        for t4 in range(IPP // 4):
            emit_km_block("y", t4)
        rs_all = singles.tile([P, 2], f32, tag="rs_all")
        cols_per_chunk = {"tts": 1, "tts2k": 1, "ttr": 2, "reduce": 4}[MODE]
        for li, (lhs_km, rhs_km) in enumerate(
            [(KM["x"], KM["y"]), (KM["y"], KM["x"])]
        ):
            acc = singles.tile([P, cols_per_chunk * NCH], f32, tag=f"acc_{li}")
            for c in range(NCH):
                if li == 0 and c % 4 == 0:
                    emit_km_block("x", c // 4)
                lhsT = lhs_km[0:K, c * P : (c + 1) * P]
                if MODE == "tts2k":
                    # FD-2048 groups: one ScalarE copy + one scan per chunk
                    pa = psum.tile([P, 2 * HBANK], f32, tag="pa")
                    pb = psum.tile([P, 2 * HBANK], f32, tag="pb")
                    for j in range(8):
                        dst = pa if j < 4 else pb
                        col = (j % 4) * 512
                        nc.tensor.matmul(
                            dst[:, col : col + 512],
                            lhsT,
                            rhs_km[0:K, j * 512 : (j + 1) * 512],
                            start=True,
                            stop=True,
                        )
                    s1 = copies.tile([P, 2 * HBANK], f32, tag="s1")
                    nc.scalar.copy(s1[:], pa[:])
                    junk = scratch.tile([P, 2 * HBANK], f32, tag="junk")
                    nc.vector.tensor_tensor_scan(
                        out=junk[:],
                        data0=pb[:],
                        data1=s1[:],
                        initial=BIG,
                        op0=MIN,
                        op1=MIN,
                    )
                    nc.gpsimd.tensor_copy(
                        acc[:, c : c + 1], junk[:, 2 * HBANK - 1 : 2 * HBANK]
                    )
                    continue
                prev_junk = None
                for half in range(2):
                    p0 = psum.tile([P, HBANK], f32, tag="p0")
                    p1 = psum.tile([P, HBANK], f32, tag="p1")
                    for q in range(4):
                        j = half * 4 + q
                        dst = p0 if q < 2 else p1
                        col = (q % 2) * 512
                        nc.tensor.matmul(
                            dst[:, col : col + 512],
                            lhsT,
                            rhs_km[0:K, j * 512 : (j + 1) * 512],
                            start=True,
                            stop=True,
                        )
                    if MODE == "tts":
                        s1 = copies.tile([P, HBANK], f32, tag="s1")
                        nc.scalar.copy(s1[:], p1[:])
                        junk = scratch.tile([P, HBANK], f32, tag="junk")
                        nc.vector.tensor_tensor_scan(
                            out=junk[:],
                            data0=p0[:],
                            data1=s1[:],
                            initial=(
                                BIG if prev_junk is None
                                else prev_junk[:, HBANK - 1 : HBANK]
                            ),
                            op0=MIN,
                            op1=MIN,
                        )
                        prev_junk = junk
                        if half == 1:
                            nc.gpsimd.tensor_copy(
                                acc[:, c : c + 1], junk[:, HBANK - 1 : HBANK]
                            )
                    elif MODE == "ttr":
                        s1 = copies.tile([P, HBANK], f32, tag="s1")
                        nc.scalar.copy(s1[:], p1[:])
                        junk = scratch.tile([P, HBANK], f32, tag="junk")
                        col_i = 2 * c + half
                        nc.vector.tensor_tensor_reduce(
                            out=junk[:],
                            in0=p0[:],
                            in1=s1[:],
                            scale=1.0,
                            scalar=BIG,
                            op0=MIN,
                            op1=MIN,
                            accum_out=acc[:, col_i : col_i + 1],
                        )
                    else:
                        col_i = 4 * c + 2 * half
                        nc.vector.tensor_reduce(
                            acc[:, col_i : col_i + 1], p0[:], axis=AX, op=MIN
                        )
                        nc.vector.tensor_reduce(
                            acc[:, col_i + 1 : col_i + 2], p1[:], axis=AX, op=MIN
                        )
            # per-point min of the group columns, then per-partition sum
            if cols_per_chunk == 1:
                rm = acc
            else:
                rm = singles.tile([P, NCH], f32, tag=f"rm_{li}")
                nc.vector.tensor_reduce(
                    rm[:],
                    acc[:].rearrange("p (c h) -> p c h", h=cols_per_chunk),
                    axis=AX,
                    op=MIN,
                )
            nc.vector.tensor_reduce(rs_all[:, li : li + 1], rm[:], axis=AX, op=ADD)
        nc.sync.dma_start(out=res_d[:, :], in_=rs_all[:])

    if not nc.is_finalized():
        nc.finalize()
    return nc


def _get_nc():
    if "nc" not in _CACHE:
        _CACHE["nc"] = _build_nc()
    return _CACHE["nc"]


def _postprocess(results):
    out = np.empty(B, np.float32)
    for b in range(B):
        r = np.asarray(results[b]["res"], dtype=np.float64)  # [128, 2]
        out[b] = (r[:, 0].sum() + r[:, 1].sum()) / N
    return out


def kernel(x, y):
    from concourse.bass_utils import run_bass_kernel_spmd

    x = np.ascontiguousarray(np.asarray(x, dtype=np.float32))
    y = np.ascontiguousarray(np.asarray(y, dtype=np.float32))
    assert x.shape == (B, N, D) and y.shape == (B, N, D)
    nc = _get_nc()
    in_maps = [{"x": x[b], "y": y[b]} for b in range(B)]
    res = run_bass_kernel_spmd(nc, in_maps, core_ids=list(range(B)))
    return _postprocess(res.results)


def timed_run(x, y, **kwargs):
    """Run with NTFF tracing; returns (output, exec_time_ns)."""
    from concourse.bass_utils import run_bass_kernel_spmd

    x = np.ascontiguousarray(np.asarray(x, dtype=np.float32))
    y = np.ascontiguousarray(np.asarray(y, dtype=np.float32))
    nc = _get_nc()
    in_maps = [{"x": x[b], "y": y[b]} for b in range(B)]
    res = run_bass_kernel_spmd(
        nc, in_maps, core_ids=list(range(B)), trace=True, **kwargs
    )
    return _postprocess(res.results), res.exec_time_ns



# revision 6
# speedup vs baseline: 1.1330x; 1.1330x over previous
"""Chamfer distance kernel for Trainium2 (8 NeuronCores, data-parallel over batch).

Full inputs x, y: [8, 4096, 3] fp32. Output: [8] fp32.

v2 strategy (per core = one batch):
  dist(i,j)*4096 computed on the PE as a K=55 fp8e4m3 DoubleRow matmul
  (0.5 cycles/output column -- 2x bf16 throughput):
    - coords scaled by sqrt(8192); 5-level fp8 splits; cross rows for all
      level pairs (i,j) with i+j<=4  -> 45 rows
    - norms scaled 4096/448, 5-level fp8 splits, paired with const-448 rows
      -> 10 rows.  Numpy-validated rel err ~1e-4 (gate 2e-2).
  Two passes (x-on-partitions then y-on-partitions); each pass streams 32
  chunks of [128 pts, 4096 cols] through a 4-slot PSUM quad ring
  ([128,1024] fp32 each).  Min-reduction is spread across all three
  consumer engines (cost-model balanced):
    - G-chunks (23/32): GpSimd tensor_tensor(min) folds quad pairs
      PSUM->SBUF; one DVE tensor_tensor_reduce finishes 2048 cols and
      drops the chunk-min into an acc column via accum_out.
    - C-chunks (9/32): ScalarE copies the 4 quads PSUM->SBUF; one DVE
      tensor_tensor_reduce over [2048]+[2048] makes the acc column.
  Final: per-pass acc [128,32] summed on DVE -> [128,2] -> DRAM; host sums
  partitions and divides by 4096*N.
"""

import os
import sys

import numpy as np

for _p in ("/opt/trn_rl_repo", "/root/.axon_site/_ro/trn_rl_repo"):
    if os.path.isdir(_p) and _p not in sys.path:
        sys.path.insert(0, _p)

B = 8
N = 4096
D = 3
P = 128
IPP = N // P      # 32 points per partition
NCH = N // P      # 32 chunks of 128 points
NLV = 5           # fp8 split levels
MAXSUM = 4        # keep cross pairs with i+j <= MAXSUM
KPART = 28        # k-rows per DoubleRow tile (2 tiles = 56 >= 55 rows)
SC = 32.0                     # coord scale; u*v = -1024*x*y = 512*(-2xy)
CONST = 128.0                 # norm pairing constant (exact in e4m3)
DSCALE = 512.0                # distance scale (= SC*SC/2); e4m3 max is 240
BIG = 3.0e38
QCOLS = 1024      # psum quad columns
N_C_CHUNKS = 9    # C-chunks per 32-chunk pass (rest are G-chunks)

_CACHE = {}

# cross pairs, ordered so lhs level i is contiguous and rhs levels 0..nr-1
# are contiguous for each group
PAIRS = []
for i in range(NLV):
    nr = 0
    for j in range(NLV):
        if i + j <= MAXSUM:
            nr += 1
    PAIRS.append((i, nr))
NCROSS = sum(nr for _, nr in PAIRS) * D  # 45
KROWS = NCROSS + 2 * NLV                 # 55


def _build_nc():
    from contextlib import ExitStack

    from concourse import bacc, mybir, masks
    from concourse.tile import TileContext

    f32 = mybir.dt.float32
    bf16 = mybir.dt.bfloat16
    fp8 = mybir.dt.float8e4
    MIN = mybir.AluOpType.min
    ADD = mybir.AluOpType.add
    AX = mybir.AxisListType.X
    DR = mybir.MatmulPerfMode.DoubleRow

    nc = bacc.Bacc()
    x_d = nc.declare_dram_parameter("x", [N, D], f32, isOutput=False)
    y_d = nc.declare_dram_parameter("y", [N, D], f32, isOutput=False)
    res_d = nc.declare_dram_parameter("res", [P, 2], f32, isOutput=True)

    with ExitStack() as ctx:
        tc = ctx.enter_context(TileContext(nc))
        singles = ctx.enter_context(tc.tile_pool(name="singles", bufs=1))
        gjp = ctx.enter_context(tc.tile_pool(name="gj", bufs=3))
        csp = ctx.enter_context(tc.tile_pool(name="cs", bufs=2))
        jkp = ctx.enter_context(tc.tile_pool(name="jk", bufs=2))
        psum = ctx.enter_context(tc.tile_pool(name="psum", bufs=4, space="PSUM"))

        # ---- PE warmup: ramp the tensor engine while DMAs/splits run.
        wsrc = singles.tile([32, 512], bf16, tag="wsrc")
        nc.gpsimd.memset(wsrc[:], 1.0)
        for wi in range(8):
            wq = psum.tile([P, QCOLS], f32, tag="q")
            nc.tensor.matmul(
                wq[:, 0:512], wsrc[:, 0:128], wsrc[:], start=True, stop=True
            )

        ident8 = singles.tile([P, P], fp8, tag="ident8")
        masks.make_identity(nc, ident8[:])

        # ---- input DMAs (points on partitions: point j = p*IPP + i)
        raw = {}
        for side, dram in (("y", y_d), ("x", x_d)):
            r = singles.tile([P, IPP, D], f32, tag=f"raw_{side}")
            nc.sync.dma_start(
                out=r[:], in_=dram[:, :].rearrange("(p i) c -> p i c", p=P)
            )
            raw[side] = r

        # ---- per-side prep: fp8 level splits + W staging [P, IPP, 56]
        # y gates pass A: its chain uses ACT casts + DVE subs; x uses Pool.
        W = {}
        for side in ("y", "x"):
            r = raw[side]
            sub_eng = nc.vector if side == "y" else nc.gpsimd
            asm_eng = nc.vector if side == "y" else nc.gpsimd
            sgn = 1.0 if side == "x" else -1.0

            # scaled coords u = sgn*SC*x   [P, IPP, D] f32
            u = singles.tile([P, IPP, D], f32, tag=f"u_{side}")
            sub_eng.tensor_scalar_mul(u[:], r[:], sgn * SC)
            # norms n = 4096*||x||^2/448  [P, IPP] f32
            sq = singles.tile([P, IPP, D], f32, tag=f"sq_{side}")
            nc.gpsimd.tensor_mul(sq[:], r[:], r[:])
            nrm = singles.tile([P, IPP], f32, tag=f"nrm_{side}")
            nc.vector.tensor_reduce(nrm[:], sq[:], axis=AX, op=ADD)
            nsc = singles.tile([P, IPP], f32, tag=f"nsc_{side}")
            sub_eng.tensor_scalar_mul(nsc[:], nrm[:], DSCALE / CONST)

            # level splits: LV [P, IPP, D, NLV] fp8, NL [P, IPP, NLV] fp8
            lv = singles.tile([P, IPP, D, NLV], fp8, tag=f"lv_{side}")
            nl = singles.tile([P, IPP, NLV], fp8, tag=f"nl_{side}")
            cur = u
            curn = nsc
            for l in range(NLV):
                nc.scalar.copy(lv[:, :, :, l], cur[:])
                nc.scalar.copy(nl[:, :, l], curn[:])
                if l < NLV - 1:
                    nxt = singles.tile([P, IPP, D], f32, tag=f"r{l}_{side}")
                    sub_eng.tensor_sub(nxt[:], cur[:], lv[:, :, :, l])
                    cur = nxt
                    nxtn = singles.tile([P, IPP], f32, tag=f"rn{l}_{side}")
                    sub_eng.tensor_sub(nxtn[:], curn[:], nl[:, :, l])
                    curn = nxtn

            # W staging [P, IPP, 56] fp8
            w = singles.tile([P, IPP, 2 * KPART], fp8, tag=f"w_{side}")
            nc.gpsimd.memset(w[:], 0.0)
            rr = 0
            for c in range(D):
                for i, nr in PAIRS:
                    if side == "x":
                        # lhs level i broadcast over nr rows
                        asm_eng.tensor_copy(
                            w[:, :, rr : rr + nr],
                            lv[:, :, c, i : i + 1].broadcast_to([P, IPP, nr]),
                        )
                    else:
                        # rhs levels 0..nr-1
                        asm_eng.tensor_copy(
                            w[:, :, rr : rr + nr], lv[:, :, c, 0:nr]
                        )
                    rr += nr
            # norm rows: 45..49 = nx levels (x side) / const (y side)
            #            50..54 = const (x side) / ny levels (y side)
            if side == "x":
                asm_eng.tensor_copy(w[:, :, NCROSS : NCROSS + NLV], nl[:])
                nc.gpsimd.memset(
                    w[:, :, NCROSS + NLV : NCROSS + 2 * NLV], CONST
                )
            else:
                nc.gpsimd.memset(w[:, :, NCROSS : NCROSS + NLV], CONST)
                asm_eng.tensor_copy(
                    w[:, :, NCROSS + NLV : NCROSS + 2 * NLV], nl[:]
                )
            W[side] = w

        # ---- KM emission: PE-transpose W blocks into PSUM (fp8), copy to
        # SBUF KM [KPART, 2, N].  One psum quad slot stages half a side
        # ([KPART, 2, N/2] fp8 = 4KB/partition).
        KM = {}
        cp_engines = [nc.scalar, nc.vector, nc.gpsimd]
        cp_i = [0]

        def emit_km(side):
            w = W[side]
            km = singles.tile([KPART, 2, N], fp8, tag=f"km_{side}")
            KM[side] = km
            for half in range(2):
                slot = psum.tile([P, QCOLS], f32, tag="q")
                tp = slot[:, :].bitcast(fp8).rearrange(
                    "p (t j) -> p t j", t=2
                )  # [128, 2, 2048] fp8 view
                for bb in range(16):
                    b = half * 16 + bb
                    for t in range(2):
                        nc.tensor.transpose(
                            tp[0:KPART, t, bb * P : (bb + 1) * P],
                            w[:, b, t * KPART : (t + 1) * KPART],
                            ident8[:],
                        )
                for cpb in range(4):
                    eng = cp_engines[cp_i[0] % 3]
                    cp_i[0] += 1
                    dst = km[
                        :, :, half * 2048 + cpb * 512 : half * 2048 + (cpb + 1) * 512
                    ]
                    src = tp[0:KPART, :, cpb * 512 : (cpb + 1) * 512]
                    if eng is nc.scalar:
                        eng.copy(dst, src)
                    else:
                        eng.tensor_copy(dst, src)

        emit_km("y")
        emit_km("x")

        # ---- main: two passes over the distance matrix
        rs_all = singles.tile([P, 2], f32, tag="rs_all")
        c_flags = [((i * N_C_CHUNKS) % NCH) < N_C_CHUNKS for i in range(NCH)]
        for li, (lhs_side, rhs_side) in enumerate((("x", "y"), ("y", "x"))):
            lhs_km, rhs_km = KM[lhs_side], KM[rhs_side]
            acc = singles.tile([P, NCH], f32, tag=f"acc_{li}")
            for c in range(NCH):
                lhsT = lhs_km[:, :, c * P : (c + 1) * P]
                quads = []
                for qi in range(4):
                    q = psum.tile([P, QCOLS], f32, tag="q")
                    quads.append(q)
                    for mj in range(2):
                        j0 = qi * QCOLS + mj * 512
                        nc.tensor.matmul(
                            q[:, mj * 512 : (mj + 1) * 512],
                            lhsT,
                            rhs_km[:, :, j0 : j0 + 512],
                            start=True,
                            stop=True,
                            perf_mode=DR,
                        )
                if c_flags[c]:
                    # C-chunk: ACT copies all quads, DVE reduces
                    cs = csp.tile([P, 4 * QCOLS], f32, tag="cs")
                    for qi in range(4):
                        nc.scalar.copy(
                            cs[:, qi * QCOLS : (qi + 1) * QCOLS], quads[qi][:]
                        )
                    junk = jkp.tile([P, 2 * QCOLS], f32, tag="junk_c")
                    nc.vector.tensor_tensor_reduce(
                        out=junk[:],
                        in0=cs[:, 0 : 2 * QCOLS],
                        in1=cs[:, 2 * QCOLS : 4 * QCOLS],
                        scale=1.0,
                        scalar=BIG,
                        op0=MIN,
                        op1=MIN,
                        accum_out=acc[:, c : c + 1],
                    )
                else:
                    # G-chunk: Pool folds quad pairs, DVE reduces
                    gj = gjp.tile([P, 2 * QCOLS], f32, tag="gj")
                    nc.gpsimd.tensor_tensor(
                        out=gj[:, 0:QCOLS], in0=quads[0][:], in1=quads[1][:], op=MIN
                    )
                    nc.gpsimd.tensor_tensor(
                        out=gj[:, QCOLS : 2 * QCOLS],
                        in0=quads[2][:],
                        in1=quads[3][:],
                        op=MIN,
                    )
                    junk = jkp.tile([P, QCOLS], f32, tag="junk_g")
                    nc.vector.tensor_tensor_reduce(
                        out=junk[:],
                        in0=gj[:, 0:QCOLS],
                        in1=gj[:, QCOLS : 2 * QCOLS],
                        scale=1.0,
                        scalar=BIG,
                        op0=MIN,
                        op1=MIN,
                        accum_out=acc[:, c : c + 1],
                    )
            nc.vector.tensor_reduce(
                rs_all[:, li : li + 1], acc[:], axis=AX, op=ADD
            )
        nc.sync.dma_start(out=res_d[:, :], in_=rs_all[:])

    if not nc.is_finalized():
        nc.finalize()
    return nc


def _get_nc():
    if "nc" not in _CACHE:
        _CACHE["nc"] = _build_nc()
    return _CACHE["nc"]


def _postprocess(results):
    out = np.empty(B, np.float32)
    for b in range(B):
        r = np.asarray(results[b]["res"], dtype=np.float64)  # [128, 2]
        out[b] = (r[:, 0].sum() + r[:, 1].sum()) / (N * DSCALE)
    return out


def kernel(x, y):
    from concourse.bass_utils import run_bass_kernel_spmd

    x = np.ascontiguousarray(np.asarray(x, dtype=np.float32))
    y = np.ascontiguousarray(np.asarray(y, dtype=np.float32))
    assert x.shape == (B, N, D) and y.shape == (B, N, D)
    nc = _get_nc()
    in_maps = [{"x": x[b], "y": y[b]} for b in range(B)]
    res = run_bass_kernel_spmd(nc, in_maps, core_ids=list(range(B)))
    return _postprocess(res.results)


def timed_run(x, y, **kwargs):
    """Run with NTFF tracing; returns (output, exec_time_ns)."""
    from concourse.bass_utils import run_bass_kernel_spmd

    x = np.ascontiguousarray(np.asarray(x, dtype=np.float32))
    y = np.ascontiguousarray(np.asarray(y, dtype=np.float32))
    nc = _get_nc()
    in_maps = [{"x": x[b], "y": y[b]} for b in range(B)]
    res = run_bass_kernel_spmd(
        nc, in_maps, core_ids=list(range(B)), trace=True, **kwargs
    )
    return _postprocess(res.results), res.exec_time_ns


# revision 7
# speedup vs baseline: 1.2455x; 1.0992x over previous
"""Chamfer distance kernel for Trainium2 (8 NeuronCores, data-parallel over batch).

Full inputs x, y: [8, 4096, 3] fp32. Output: [8] fp32.

v3 strategy (per core = one batch):
  dist(i,j)*512 computed on the PE as a K=38 fp8e4m3 DoubleRow matmul
  (0.5 cycles/output column -- 2x bf16 throughput):
    - coords scaled by 32; 4-level fp8 splits; cross rows for level pairs
      (i,j) with i+j<=3  -> 30 rows
    - norms scaled 512/128, 4-level fp8 splits, paired with const-128 rows
      -> 8 rows.  Numpy-validated rel err ~1e-4 (gate 2e-2).
  Two passes (x-on-partitions then y-on-partitions); each pass streams 32
  chunks of [128 pts, 4096 cols] through a 4-slot PSUM quad ring
  ([128,1024] fp32).  Min-reduction is split across all three consumer
  engines; C-chunks (ScalarE copies + DVE reduce) are quad-interleaved
  with a partner G-chunk (GpSimd pair-folds + DVE reduce) so ScalarE and
  GpSimd stream concurrently instead of alternating:
    - G-chunks (23/32): gpsimd tensor_tensor(min) folds quad pairs
      PSUM->SBUF; one DVE tensor_tensor_reduce finishes 2048 cols and
      drops the chunk min into an acc column via accum_out.
    - C-chunks (9/32): ScalarE copies the 4 quads PSUM->SBUF; one DVE
      tensor_tensor_reduce over [2048]+[2048] makes the acc column.
  Final: per-pass acc [128,32] summed on DVE -> [128,2] -> DRAM; host sums
  partitions and divides by 512*N.
"""

import os
import sys

import numpy as np

for _p in ("/opt/trn_rl_repo", "/root/.axon_site/_ro/trn_rl_repo"):
    if os.path.isdir(_p) and _p not in sys.path:
        sys.path.insert(0, _p)

B = 8
N = 4096
D = 3
P = 128
IPP = N // P      # 32 points per partition
NCH = N // P      # 32 chunks of 128 points
NLV = 4           # fp8 split levels
MAXSUM = 3        # keep cross pairs with i+j <= MAXSUM
SC = 32.0         # coord scale; u*v = -1024*x*y = 512*(-2xy)
CONST = 128.0     # norm pairing constant (exact in e4m3)
DSCALE = 512.0    # distance scale (= SC*SC/2); e4m3 max is 240
BIG = 3.0e38
QCOLS = 1024      # psum quad columns
N_C_CHUNKS = 9    # C-chunks per 32-chunk pass (rest are G-chunks)

_CACHE = {}

# cross pairs (i = lhs level, contiguous rhs levels 0..nr-1 per group)
PAIRS = []
for i in range(NLV):
    nr = sum(1 for j in range(NLV) if i + j <= MAXSUM)
    if nr:
        PAIRS.append((i, nr))
NCROSS = sum(nr for _, nr in PAIRS) * D   # 30
KROWS = NCROSS + 2 * NLV                  # 38
KPART = KROWS // 2                        # 19 rows per DoubleRow k-tile


def _build_nc():
    from contextlib import ExitStack

    from concourse import bacc, mybir, masks
    from concourse.tile import TileContext

    f32 = mybir.dt.float32
    bf16 = mybir.dt.bfloat16
    fp8 = mybir.dt.float8e4
    MIN = mybir.AluOpType.min
    ADD = mybir.AluOpType.add
    AX = mybir.AxisListType.X
    DR = mybir.MatmulPerfMode.DoubleRow

    nc = bacc.Bacc()
    x_d = nc.declare_dram_parameter("x", [N, D], f32, isOutput=False)
    y_d = nc.declare_dram_parameter("y", [N, D], f32, isOutput=False)
    res_d = nc.declare_dram_parameter("res", [P, 2], f32, isOutput=True)

    with ExitStack() as ctx:
        tc = ctx.enter_context(TileContext(nc))
        singles = ctx.enter_context(tc.tile_pool(name="singles", bufs=1))
        gjp = ctx.enter_context(tc.tile_pool(name="gj", bufs=3))
        csp = ctx.enter_context(tc.tile_pool(name="cs", bufs=2))
        jkp = ctx.enter_context(tc.tile_pool(name="jk", bufs=2))
        psum = ctx.enter_context(tc.tile_pool(name="psum", bufs=4, space="PSUM"))

        # ---- PE warmup: ramp the tensor engine while DMAs/splits run.
        wsrc = singles.tile([32, 512], bf16, tag="wsrc")
        nc.gpsimd.memset(wsrc[:], 1.0)
        for wi in range(8):
            wq = psum.tile([P, QCOLS], f32, tag="q")
            nc.tensor.matmul(
                wq[:, 0:512], wsrc[:, 0:128], wsrc[:], start=True, stop=True
            )

        ident8 = singles.tile([P, P], fp8, tag="ident8")
        masks.make_identity(nc, ident8[:])

        # ---- input DMAs (points on partitions: point j = p*IPP + i)
        raw = {}
        for side, dram in (("y", y_d), ("x", x_d)):
            r = singles.tile([P, IPP, D], f32, tag=f"raw_{side}")
            nc.sync.dma_start(
                out=r[:], in_=dram[:, :].rearrange("(p i) c -> p i c", p=P)
            )
            raw[side] = r

        # ---- per-side prep: fp8 level splits + W staging [P, IPP, 38]
        # y gates pass A: its chain uses ACT casts + DVE subs; x uses Pool.
        W = {}
        for side in ("y", "x"):
            r = raw[side]
            sub_eng = nc.vector if side == "y" else nc.gpsimd
            asm_eng = nc.vector if side == "y" else nc.gpsimd
            sgn = 1.0 if side == "x" else -1.0

            u = singles.tile([P, IPP, D], f32, tag=f"u_{side}")
            sub_eng.tensor_scalar_mul(u[:], r[:], sgn * SC)
            sq = singles.tile([P, IPP, D], f32, tag=f"sq_{side}")
            nc.gpsimd.tensor_mul(sq[:], r[:], r[:])
            nrm = singles.tile([P, IPP], f32, tag=f"nrm_{side}")
            nc.vector.tensor_reduce(nrm[:], sq[:], axis=AX, op=ADD)
            nsc = singles.tile([P, IPP], f32, tag=f"nsc_{side}")
            sub_eng.tensor_scalar_mul(nsc[:], nrm[:], DSCALE / CONST)

            lv = singles.tile([P, IPP, D, NLV], fp8, tag=f"lv_{side}")
            nl = singles.tile([P, IPP, NLV], fp8, tag=f"nl_{side}")
            cur = u
            curn = nsc
            for l in range(NLV):
                nc.scalar.copy(lv[:, :, :, l], cur[:])
                nc.scalar.copy(nl[:, :, l], curn[:])
                if l < NLV - 1:
                    nxt = singles.tile([P, IPP, D], f32, tag=f"r{l}_{side}")
                    sub_eng.tensor_sub(nxt[:], cur[:], lv[:, :, :, l])
                    cur = nxt
                    nxtn = singles.tile([P, IPP], f32, tag=f"rn{l}_{side}")
                    sub_eng.tensor_sub(nxtn[:], curn[:], nl[:, :, l])
                    curn = nxtn

            w = singles.tile([P, IPP, KROWS], fp8, tag=f"w_{side}")
            rr = 0
            for c in range(D):
                for i, nr in PAIRS:
                    if side == "x":
                        asm_eng.tensor_copy(
                            w[:, :, rr : rr + nr],
                            lv[:, :, c, i : i + 1].broadcast_to([P, IPP, nr]),
                        )
                    else:
                        asm_eng.tensor_copy(
                            w[:, :, rr : rr + nr], lv[:, :, c, 0:nr]
                        )
                    rr += nr
            if side == "x":
                asm_eng.tensor_copy(w[:, :, NCROSS : NCROSS + NLV], nl[:])
                nc.gpsimd.memset(w[:, :, NCROSS + NLV : KROWS], CONST)
            else:
                nc.gpsimd.memset(w[:, :, NCROSS : NCROSS + NLV], CONST)
                asm_eng.tensor_copy(w[:, :, NCROSS + NLV : KROWS], nl[:])
            W[side] = w

        # ---- KM emission: PE-transpose W blocks into PSUM (fp8), copy to
        # SBUF KM [KPART, 2, N].  One psum quad slot stages half a side.
        KM = {}
        cp_engines = [nc.scalar, nc.vector, nc.gpsimd]
        cp_i = [0]

        def emit_km(side):
            w = W[side]
            km = singles.tile([KPART, 2, N], fp8, tag=f"km_{side}")
            KM[side] = km
            for half in range(2):
                slot = psum.tile([P, QCOLS], f32, tag="q")
                tp = slot[:, :].bitcast(fp8).rearrange(
                    "p (t j) -> p t j", t=2
                )  # [128, 2, 2048] fp8 view
                for bb in range(16):
                    b = half * 16 + bb
                    for t in range(2):
                        nc.tensor.transpose(
                            tp[0:KPART, t, bb * P : (bb + 1) * P],
                            w[:, b, t * KPART : (t + 1) * KPART],
                            ident8[:],
                        )
                for cpb in range(4):
                    eng = cp_engines[cp_i[0] % 3]
                    cp_i[0] += 1
                    dst = km[
                        :, :,
                        half * 2048 + cpb * 512 : half * 2048 + (cpb + 1) * 512,
                    ]
                    src = tp[0:KPART, :, cpb * 512 : (cpb + 1) * 512]
                    if eng is nc.scalar:
                        eng.copy(dst, src)
                    else:
                        eng.tensor_copy(dst, src)

        emit_km("y")
        emit_km("x")

        # ---- main: two passes; C-chunks quad-interleaved with partner Gs
        rs_all = singles.tile([P, 2], f32, tag="rs_all")
        c_flags = [((i * N_C_CHUNKS) % NCH) < N_C_CHUNKS for i in range(NCH)]
        units = []  # ("G", c) or ("CG", c_c, c_g)
        ci = 0
        while ci < NCH:
            if c_flags[ci] and ci + 1 < NCH and not c_flags[ci + 1]:
                units.append(("CG", ci, ci + 1))
                ci += 2
            elif c_flags[ci]:
                units.append(("C", ci))
                ci += 1
            else:
                units.append(("G", ci))
                ci += 1

        for li, (lhs_side, rhs_side) in enumerate((("x", "y"), ("y", "x"))):
            lhs_km, rhs_km = KM[lhs_side], KM[rhs_side]
            acc = singles.tile([P, NCH], f32, tag=f"acc_{li}")

            def mm(q, lhsT, qi, mj):
                j0 = qi * QCOLS + mj * 512
                nc.tensor.matmul(
                    q[:, mj * 512 : (mj + 1) * 512],
                    lhsT,
                    rhs_km[:, :, j0 : j0 + 512],
                    start=True,
                    stop=True,
                    perf_mode=DR,
                )

            def consume_g(quads, c):
                gj = gjp.tile([P, 2 * QCOLS], f32, tag="gj")
                nc.gpsimd.tensor_tensor(
                    out=gj[:, 0:QCOLS], in0=quads[0][:], in1=quads[1][:], op=MIN
                )
                nc.gpsimd.tensor_tensor(
                    out=gj[:, QCOLS:], in0=quads[2][:], in1=quads[3][:], op=MIN
                )
                junk = jkp.tile([P, QCOLS], f32, tag="junk_g")
                nc.vector.tensor_tensor_reduce(
                    out=junk[:],
                    in0=gj[:, 0:QCOLS],
                    in1=gj[:, QCOLS:],
                    scale=1.0,
                    scalar=BIG,
                    op0=MIN,
                    op1=MIN,
                    accum_out=acc[:, c : c + 1],
                )

            def consume_c_final(cs, c):
                junk = jkp.tile([P, 2 * QCOLS], f32, tag="junk_c")
                nc.vector.tensor_tensor_reduce(
                    out=junk[:],
                    in0=cs[:, 0 : 2 * QCOLS],
                    in1=cs[:, 2 * QCOLS :],
                    scale=1.0,
                    scalar=BIG,
                    op0=MIN,
                    op1=MIN,
                    accum_out=acc[:, c : c + 1],
                )

            for unit in units:
                if unit[0] == "CG":
                    _, cc, cg = unit
                    lhsT_c = lhs_km[:, :, cc * P : (cc + 1) * P]
                    lhsT_g = lhs_km[:, :, cg * P : (cg + 1) * P]
                    cs = csp.tile([P, 4 * QCOLS], f32, tag="cs")
                    gq = []
                    for qi in range(4):
                        qc = psum.tile([P, QCOLS], f32, tag="q")
                        for mj in range(2):
                            mm(qc, lhsT_c, qi, mj)
                        qg = psum.tile([P, QCOLS], f32, tag="q")
                        for mj in range(2):
                            mm(qg, lhsT_g, qi, mj)
                        gq.append(qg)
                        nc.scalar.copy(
                            cs[:, qi * QCOLS : (qi + 1) * QCOLS], qc[:]
                        )
                    consume_g(gq, cg)
                    consume_c_final(cs, cc)
                else:
                    kind, c = unit[0], unit[1]
                    lhsT = lhs_km[:, :, c * P : (c + 1) * P]
                    quads = []
                    for qi in range(4):
                        q = psum.tile([P, QCOLS], f32, tag="q")
                        quads.append(q)
                        for mj in range(2):
                            mm(q, lhsT, qi, mj)
                    if kind == "C":
                        cs = csp.tile([P, 4 * QCOLS], f32, tag="cs")
                        for qi in range(4):
                            nc.scalar.copy(
                                cs[:, qi * QCOLS : (qi + 1) * QCOLS],
                                quads[qi][:],
                            )
                        consume_c_final(cs, c)
                    else:
                        consume_g(quads, c)
            nc.vector.tensor_reduce(
                rs_all[:, li : li + 1], acc[:], axis=AX, op=ADD
            )
        nc.sync.dma_start(out=res_d[:, :], in_=rs_all[:])

    if not nc.is_finalized():
        nc.finalize()
    return nc


def _get_nc():
    if "nc" not in _CACHE:
        _CACHE["nc"] = _build_nc()
    return _CACHE["nc"]


def _postprocess(results):
    out = np.empty(B, np.float32)
    for b in range(B):
        r = np.asarray(results[b]["res"], dtype=np.float64)  # [128, 2]
        out[b] = (r[:, 0].sum() + r[:, 1].sum()) / (N * DSCALE)
    return out


def kernel(x, y):
    from concourse.bass_utils import run_bass_kernel_spmd

    x = np.ascontiguousarray(np.asarray(x, dtype=np.float32))
    y = np.ascontiguousarray(np.asarray(y, dtype=np.float32))
    assert x.shape == (B, N, D) and y.shape == (B, N, D)
    nc = _get_nc()
    in_maps = [{"x": x[b], "y": y[b]} for b in range(B)]
    res = run_bass_kernel_spmd(nc, in_maps, core_ids=list(range(B)))
    return _postprocess(res.results)


def timed_run(x, y, **kwargs):
    """Run with NTFF tracing; returns (output, exec_time_ns)."""
    from concourse.bass_utils import run_bass_kernel_spmd

    x = np.ascontiguousarray(np.asarray(x, dtype=np.float32))
    y = np.ascontiguousarray(np.asarray(y, dtype=np.float32))
    nc = _get_nc()
    in_maps = [{"x": x[b], "y": y[b]} for b in range(B)]
    res = run_bass_kernel_spmd(
        nc, in_maps, core_ids=list(range(B)), trace=True, **kwargs
    )
    return _postprocess(res.results), res.exec_time_ns
